# revision 12
# baseline (speedup 1.0000x reference)
"""Trainium2 Bass kernel for nn_DAttention:
out[b,c,d,h,w] = x[b,c,d,h,w] * mean_{c,h,w}(x[b,:,d,:,:]).

Sharding: pure data parallel over batch B=8 -> one batch per NeuronCore
(x[b] is a contiguous zero-copy slice). Per core, loop over 16 groups of
TWO adjacent d-slices (4 MiB each): load into SBUF, reduce each d's
524288 elements to its mean, multiply, store. Single pass over HBM:
64 MiB read (fp32) + 32 MiB written (bf16) per core.

SBUF layout per d-pair: tile [128, 8192] with partition p = c*4 +
(d%2)*2 + (h//64), free = (h%64)*128 + w. Each partition row is one
contiguous 32 KiB DRAM run (16 KiB bf16 on the store side) -> one
descriptor per partition at SDMA line rate. Halving the descriptor
count vs per-d tiles matters because the slowest SDMA engine (#15, a
known trn2 quirk) pays ~135 ns of per-descriptor overhead on loads and
is 100% busy — it IS the critical path.

Engine schedule per group (exactly ONE big op on DVE — adding a
tensor_reduce on DVE couples the load stream to DVE's program order
and stalls it ~15 us every few groups; measured +50 us):
  ACT: four activation-Copies (2048-col chunks) into a dead PSUM
       scratch with accum_out -> per-partition column sums
  PE : four accumulated matmuls against a constant 128x128 selection
       matrix S[k,p] = 1/524288 iff partitions k,p hold the same d
       -> per-d sum + broadcast of each d's mean to its 64 partitions
  DVE: single tensor_scalar multiply (fp32 in, bf16 out) reading the
       mean scalars directly from PSUM
  ACT: store DMA issue (keep sel's load on the sync ring — moving it
       to the scalar ring perturbs queue allocation)

The store is bf16 (products span ~1e-7..3e-2; bf16 keeps 2^-8 relative
accuracy at every magnitude — no fp16 subnormal cliff). The host casts
bf16 -> fp32 on gather (value-preserving). Relative error 1.96e-3 vs
the 2e-2 gate.

Measured per-core exec: 254-256 us on a quiet core (engines at exact
SDMA line rate, 98% of the 435 GB/s per-core fabric ceiling; absolute
floor for 96 MiB is ~231 us + ~15 us edges). Cores with the episodic
engine-15 slowdown measure ~280-307 us; that degradation is per-byte,
so no descriptor-size choice avoids it.
"""
import numpy as np

import concourse.bacc as bacc
import concourse.tile as tile
import concourse.mybir as mybir
from concourse.bass_utils import run_bass_kernel_spmd

B, C, D, H, W = 8, 32, 32, 128, 128
DG, DJ = 16, 2          # D split: groups of 2 adjacent d
HH, HL = 2, 64          # H split: partition dim = C*DJ*HH = 128
P = C * DJ * HH         # 128 partitions
F = HL * W              # 8192 free elements per partition
NCH = 4                 # ACT reduce chunks per group (PSUM scratch cols)
N_RED = C * H * W       # 524288 = 2**19 elements reduced per (b, d)
RECIP = 1.0 / N_RED     # exact in fp32

_NC = None


def _sel_matrix() -> np.ndarray:
    """S[k,p] = RECIP iff partitions k and p belong to the same d."""
    p = np.arange(P)
    dj = (p >> 1) & 1
    return (dj[:, None] == dj[None, :]).astype(np.float32) * np.float32(RECIP)


def _build_nc(xin_bufs=4, out_bufs=3):
    nc = bacc.Bacc("TRN2", target_bir_lowering=False, debug=False)
    x6 = nc.dram_tensor(
        "x", [C, DG, DJ, HH, HL, W], mybir.dt.float32, kind="ExternalInput"
    )
    o6 = nc.dram_tensor(
        "out", [C, DG, DJ, HH, HL, W], mybir.dt.bfloat16, kind="ExternalOutput"
    )
    sel = nc.dram_tensor("sel", [P, P], mybir.dt.float32, kind="ExternalInput")
    chunk = F // NCH
    with tile.TileContext(nc) as tc:
        with (
            tc.tile_pool(name="xin", bufs=xin_bufs) as xpool,
            tc.tile_pool(name="oout", bufs=out_bufs) as opool,
            tc.tile_pool(name="small", bufs=2 * NCH) as spool,
            tc.tile_pool(name="psum", bufs=2, space="PSUM") as ppool,
            tc.tile_pool(name="psc", bufs=1, space="PSUM") as scpool,
            tc.tile_pool(name="const", bufs=1) as cpool,
        ):
            selt = cpool.tile([P, P], mybir.dt.float32)
            for g in range(DG):
                xt = xpool.tile([P, F], mybir.dt.float32, tag="xt")
                nc.sync.dma_start(xt[:], x6[:, g])
                if g == 0:
                    # after the first big load issue: the stream starts
                    # ~1.3 us earlier and sel still lands well before
                    # the first matmul reads it
                    nc.sync.dma_start(selt[:], sel[:])
                scratch = scpool.tile([P, chunk], mybir.dt.float32, tag="sc")
                dv = ppool.tile([P, 1], mybir.dt.float32, tag="dv")
                for k in range(NCH):
                    cs = spool.tile([P, 1], mybir.dt.float32, tag=f"cs{k}")
                    nc.scalar.activation(
                        scratch[:], xt[:, k * chunk:(k + 1) * chunk],
                        mybir.ActivationFunctionType.Copy, accum_out=cs[:],
                    )
                    nc.tensor.matmul(
                        dv[:], selt[:], cs[:],
                        start=(k == 0), stop=(k == NCH - 1),
                    )
                ot = opool.tile([P, F], mybir.dt.bfloat16, tag="ot")
                if g < DG - 1:
                    nc.vector.tensor_scalar_mul(ot[:], xt[:], dv[:])
                    nc.scalar.dma_start(o6[:, g], ot[:])
                else:
                    # final group: split the multiply+store so the first
                    # half's store streams while the second half computes,
                    # shortening the pipeline drain
                    half = F // 2
                    nc.vector.tensor_scalar_mul(
                        ot[:, :half], xt[:, :half], dv[:]
                    )
                    nc.scalar.dma_start(o6[:, g, :, :, :HL // 2], ot[:, :half])
                    nc.vector.tensor_scalar_mul(
                        ot[:, half:], xt[:, half:], dv[:]
                    )
                    nc.scalar.dma_start(o6[:, g, :, :, HL // 2:], ot[:, half:])
    nc.compile()
    return nc


def _get_nc():
    global _NC
    if _NC is None:
        _NC = _build_nc()
    return _NC


def run(x: np.ndarray, trace: bool = False, tmpdir: str | None = None):
    """Run on 8 NeuronCores; returns (out, BassKernelResults)."""
    x = np.asarray(x)
    assert x.shape == (B, C, D, H, W), x.shape
    x = x.astype(np.float32, copy=False)
    nc = _get_nc()
    S = _sel_matrix()
    in_maps = [
        {
            "x": np.ascontiguousarray(x[b]).reshape(C, DG, DJ, HH, HL, W),
            "sel": S,
        }
        for b in range(B)
    ]
    res = run_bass_kernel_spmd(
        nc, in_maps, core_ids=list(range(B)), trace=trace, tmpdir=tmpdir
    )
    out = np.stack(
        [r["out"].reshape(C, D, H, W).astype(np.float32) for r in res.results]
    )
    return out, res


def kernel(x: np.ndarray) -> np.ndarray:
    out, _ = run(x)
    return out


# revision 13
# speedup vs baseline: 1.1910x; 1.1910x over previous
"""Trainium2 Bass kernel for nn_DAttention:
out[b,c,d,h,w] = x[b,c,d,h,w] * mean_{c,h,w}(x[b,:,d,:,:]).

Sharding: pure data parallel over batch B=8 -> one batch per NeuronCore
(x[b] is a contiguous zero-copy slice). Per core, loop over 16 groups of
TWO adjacent d-slices (4 MiB each): load into SBUF, reduce each d's
524288 elements to its mean, multiply, store. Single pass over HBM:
64 MiB read (fp32) + 32 MiB written (bf16) per core.

SBUF layout per d-pair: tile [128, 8192] with partition p = c*4 +
(d%2)*2 + (h//64), free = (h%64)*128 + w. Each partition row is one
contiguous 32 KiB DRAM run (16 KiB bf16 on the store side) -> one
descriptor per partition at SDMA line rate. Halving the descriptor
count vs per-d tiles matters because the slowest SDMA engine (#15, a
known trn2 quirk) pays ~135 ns of per-descriptor overhead on loads and
is 100% busy — it IS the critical path.

Engine schedule per group (exactly ONE big op on DVE — adding a
tensor_reduce on DVE couples the load stream to DVE's program order
and stalls it ~15 us every few groups; measured +50 us):
  ACT: four activation-Copies (2048-col chunks) into a dead PSUM
       scratch with accum_out -> per-partition column sums
  PE : four accumulated matmuls against a constant 128x128 selection
       matrix S[k,p] = 1/524288 iff partitions k,p hold the same d
       -> per-d sum + broadcast of each d's mean to its 64 partitions
  DVE: single tensor_scalar multiply (fp32 in, bf16 out) reading the
       mean scalars directly from PSUM
  ACT: store DMA issue (keep sel's load on the sync ring — moving it
       to the scalar ring perturbs queue allocation)

The store is bf16 (products span ~1e-7..3e-2; bf16 keeps 2^-8 relative
accuracy at every magnitude — no fp16 subnormal cliff). The host casts
bf16 -> fp32 on gather (value-preserving). Relative error 1.96e-3 vs
the 2e-2 gate.

Measured per-core exec: 254-256 us on a quiet core (engines at exact
SDMA line rate, 98% of the 435 GB/s per-core fabric ceiling; absolute
floor for 96 MiB is ~231 us + ~15 us edges). Cores with the episodic
engine-15 slowdown measure ~280-307 us; that degradation is per-byte,
so no descriptor-size choice avoids it. Two edge trims (trace-verified):
issuing sel's load after the first xt load pulls the stream start from
t=8.4 to t=5.9 us, and splitting the final group's multiply+store
removes the tail gaps (last stores drain back-to-back at line rate).
"""
import numpy as np

import concourse.bacc as bacc
import concourse.tile as tile
import concourse.mybir as mybir
from concourse.bass_utils import run_bass_kernel_spmd

B, C, D, H, W = 8, 32, 32, 128, 128
DG, DJ = 16, 2          # D split: groups of 2 adjacent d
HH, HL = 2, 64          # H split: partition dim = C*DJ*HH = 128
P = C * DJ * HH         # 128 partitions
F = HL * W              # 8192 free elements per partition
NCH = 4                 # ACT reduce chunks per group (PSUM scratch cols)
N_RED = C * H * W       # 524288 = 2**19 elements reduced per (b, d)
RECIP = 1.0 / N_RED     # exact in fp32

_NC = None


def _sel_matrix() -> np.ndarray:
    """S[k,p] = RECIP iff partitions k and p belong to the same d."""
    p = np.arange(P)
    dj = (p >> 1) & 1
    return (dj[:, None] == dj[None, :]).astype(np.float32) * np.float32(RECIP)


def _build_nc(xin_bufs=4, out_bufs=3):
    nc = bacc.Bacc("TRN2", target_bir_lowering=False, debug=False)
    x6 = nc.dram_tensor(
        "x", [C, DG, DJ, HH, HL, W], mybir.dt.float32, kind="ExternalInput"
    )
    o6 = nc.dram_tensor(
        "out", [C, DG, DJ, HH, HL, W], mybir.dt.bfloat16, kind="ExternalOutput"
    )
    sel = nc.dram_tensor("sel", [P, P], mybir.dt.float32, kind="ExternalInput")
    chunk = F // NCH
    with tile.TileContext(nc) as tc:
        with (
            tc.tile_pool(name="xin", bufs=xin_bufs) as xpool,
            tc.tile_pool(name="oout", bufs=out_bufs) as opool,
            tc.tile_pool(name="small", bufs=2 * NCH) as spool,
            tc.tile_pool(name="psum", bufs=2, space="PSUM") as ppool,
            tc.tile_pool(name="psc", bufs=1, space="PSUM") as scpool,
            tc.tile_pool(name="const", bufs=1) as cpool,
        ):
            selt = cpool.tile([P, P], mybir.dt.float32)
            for g in range(DG):
                xt = xpool.tile([P, F], mybir.dt.float32, tag="xt")
                nc.sync.dma_start(xt[:], x6[:, g])
                if g == 0:
                    # after the first big load issue: the stream starts
                    # ~1.3 us earlier and sel still lands well before
                    # the first matmul reads it
                    nc.sync.dma_start(selt[:], sel[:])
                scratch = scpool.tile([P, chunk], mybir.dt.float32, tag="sc")
                dv = ppool.tile([P, 1], mybir.dt.float32, tag="dv")
                for k in range(NCH):
                    cs = spool.tile([P, 1], mybir.dt.float32, tag=f"cs{k}")
                    nc.scalar.activation(
                        scratch[:], xt[:, k * chunk:(k + 1) * chunk],
                        mybir.ActivationFunctionType.Copy, accum_out=cs[:],
                    )
                    nc.tensor.matmul(
                        dv[:], selt[:], cs[:],
                        start=(k == 0), stop=(k == NCH - 1),
                    )
                ot = opool.tile([P, F], mybir.dt.bfloat16, tag="ot")
                if g < DG - 1:
                    nc.vector.tensor_scalar_mul(ot[:], xt[:], dv[:])
                    nc.scalar.dma_start(o6[:, g], ot[:])
                else:
                    # final group: split the multiply+store so the first
                    # half's store streams while the second half computes,
                    # shortening the pipeline drain
                    half = F // 2
                    nc.vector.tensor_scalar_mul(
                        ot[:, :half], xt[:, :half], dv[:]
                    )
                    nc.scalar.dma_start(o6[:, g, :, :, :HL // 2], ot[:, :half])
                    nc.vector.tensor_scalar_mul(
                        ot[:, half:], xt[:, half:], dv[:]
                    )
                    nc.scalar.dma_start(o6[:, g, :, :, HL // 2:], ot[:, half:])
    nc.compile()
    return nc


def _get_nc():
    global _NC
    if _NC is None:
        _NC = _build_nc()
    return _NC


def run(x: np.ndarray, trace: bool = False, tmpdir: str | None = None):
    """Run on 8 NeuronCores; returns (out, BassKernelResults)."""
    x = np.asarray(x)
    assert x.shape == (B, C, D, H, W), x.shape
    x = x.astype(np.float32, copy=False)
    nc = _get_nc()
    S = _sel_matrix()
    in_maps = [
        {
            "x": np.ascontiguousarray(x[b]).reshape(C, DG, DJ, HH, HL, W),
            "sel": S,
        }
        for b in range(B)
    ]
    res = run_bass_kernel_spmd(
        nc, in_maps, core_ids=list(range(B)), trace=trace, tmpdir=tmpdir
    )
    out = np.stack(
        [r["out"].reshape(C, D, H, W).astype(np.float32) for r in res.results]
    )
    return out, res


def kernel(x: np.ndarray) -> np.ndarray:
    out, _ = run(x)
    return out
